# revision 22
# baseline (speedup 1.0000x reference)
"""Trainium2 Bass kernel for nn_AttentionMechanism (dense_transformer).

Reference math (per batch b):
    context_proj = einsum('bdc,hd->bch', cv, W) + bias        # [B,C,H]
    scores       = einsum('bch,bh->bc', context_proj, hidden) # [B,C]
    attn         = softmax(scores, axis=1)
    ctx          = einsum('bdc,bc->bd', cv, attn)             # [B,D]
    out          = broadcast(ctx, (seqlen, B, D))

Algebraic simplification: scores[b,c] = sum_d cv[b,d,c]*v[b,d] + const(b)
with v = hidden @ W; the constant cancels in softmax so the bias vector is
dropped entirely.

Device pipeline (per core, 4 batches, everything fully unrolled):
  - cv is DMA-cast-loaded (SWDGE) straight into float32r tiles (free
    TF32-like rounding, ~1.2e-4 rel) so both big contractions run on the
    TensorEngine at 1 cycle/row instead of fp32's 4.
  - scores: lhsT is the v-column replicated across 128 free positions, so
    the PSUM score banks come out partition-replicated and the whole
    softmax runs per-partition with zero cross-partition ops.
  - softmax: DVE reduce_max (negated) -> ACT Exp with accum_out (fused
    sum of exponentials) -> reciprocal.
  - ctx: cv tiles are PE-transposed (f32r, 1.5 cyc/row) into [c,d] layout
    via PSUM, copied to SBUF (DVE/ACT split), then contracted against the
    partition-replicated transposed weights pT on the PE.
  - out[t, b, :] = ctx row, written with a stride-0-replicated DMA.

Sharding: data-parallel over batch, 4 batches per core on 8 NeuronCores.
"""

import sys

if "/opt/trn_rl_repo" not in sys.path:
    sys.path.insert(0, "/opt/trn_rl_repo")

import numpy as np

# Problem constants (hardcoded; kernel.py must be self-contained).
B = 32
N_CORES = 8
BL = B // N_CORES   # 4 batches per core
D = 1024
C = 2048
H = 1024
SEQ = 64
P = 128
DT = D // P         # 8 d-tiles
HT = H // P         # 8 h-tiles
CCH = 512           # one fp32 PSUM bank
NJ = C // CCH       # 4 c-chunks
NG = C // P         # 16 c-tiles (transpose groups)

_NC_CACHE = {}


def _build_nc():
    import concourse.bass as bass
    import concourse.mybir as mybir
    from concourse.bacc import Bacc
    from concourse.tile import TileContext
    from contextlib import ExitStack

    fp32 = mybir.dt.float32
    f32r = mybir.dt.float32r
    AF = mybir.ActivationFunctionType
    AX = mybir.AxisListType

    nc = Bacc("TRN2")

    cv_t = nc.dram_tensor("cv", [BL, D, C], fp32, kind="ExternalInput")
    hT_t = nc.dram_tensor("hT", [H, BL], fp32, kind="ExternalInput")
    w_t = nc.dram_tensor("W", [H, D], fp32, kind="ExternalInput")
    id_t = nc.dram_tensor("ident", [P, P], fp32, kind="ExternalInput")
    out_t = nc.dram_tensor("out", [SEQ, BL, D], fp32, kind="ExternalOutput")

    with ExitStack() as ctx:
        tc = ctx.enter_context(TileContext(nc))

        singles = ctx.enter_context(tc.tile_pool(name="singles", bufs=1))
        wpool = ctx.enter_context(tc.tile_pool(name="wpool", bufs=2))
        cvpool = ctx.enter_context(tc.tile_pool(name="cvpool", bufs=14))
        ppool = ctx.enter_context(tc.tile_pool(name="ppool", bufs=2))
        ptpool = ctx.enter_context(tc.tile_pool(name="ptpool", bufs=2))
        cvtpool = ctx.enter_context(tc.tile_pool(name="cvtpool", bufs=6))
        small = ctx.enter_context(tc.tile_pool(name="small", bufs=8))
        rowpool = ctx.enter_context(tc.tile_pool(name="rowpool", bufs=2))
        psum = ctx.enter_context(tc.tile_pool(name="psum", bufs=8, space="PSUM"))

        # ---- constants -------------------------------------------------
        ident_f = singles.tile([P, P], fp32)
        nc.sync.dma_start(out=ident_f[:, :], in_=id_t[:, :])
        ident_r = singles.tile([P, P], f32r)
        nc.gpsimd.dma_start(out=ident_r[:, :], in_=id_t[:, :])

        # hT dram [H, BL] -> sbuf [h_lo(128), ht(8), b(4)]
        hT_sb = singles.tile([P, HT, BL], fp32)
        hT_ap = hT_t[:, :].rearrange("(ht p) b -> p ht b", p=P)
        nc.sync.dma_start(out=hT_sb[:, :, :], in_=hT_ap)

        # ---- phase 0: vT = W-contraction with hidden -------------------
        # pv[dt][d_lo, b] = sum_h W[h, dt*128+d_lo] * hidden[b, h]
        pv = [psum.tile([P, CCH], fp32, tag="bank", name=f"pv{i}") for i in range(DT)]
        for ht in range(HT):
            w_sb = wpool.tile([P, D], fp32, tag="w")
            nc.sync.dma_start(out=w_sb[:, :], in_=w_t[ht * P : (ht + 1) * P, :])
            for dt in range(DT):
                nc.tensor.matmul(
                    pv[dt][:, :BL],
                    lhsT=w_sb[:, dt * P : (dt + 1) * P],
                    rhs=hT_sb[:, ht, :],
                    start=(ht == 0),
                    stop=(ht == HT - 1),
                )
        # vT_rep[:, dt*512 + b*128 + r] = v[dt*128+d_lo, b] for all r
        # (fp32 PSUM -> f32r SBUF copy does the fp32r rounding)
        # vT_err_rep carries the f32r-rounded residual v - round(v) so the
        # scores matmul can run as a two-term f32r split (v-side exact to
        # ~2^-24), halving the end-to-end error vs single-term f32r.
        vT_rep = singles.tile([P, DT * CCH], f32r)
        vT_err_rep = singles.tile([P, DT * CCH], f32r)
        for dt in range(DT):
            src = pv[dt][:, :BL]
            rep_src = bass.AP(
                tensor=src.tensor,
                offset=src.offset,
                ap=[src.ap[0], [src.ap[-1][0], BL], [0, P]],
            )
            nc.vector.tensor_copy(
                out=vT_rep[:, dt * CCH : (dt + 1) * CCH], in_=rep_src
            )
            verr = small.tile([P, BL], fp32, tag="verr", name=f"verr{dt}")
            vr_slice = bass.AP(
                tensor=vT_rep.tensor,
                offset=vT_rep.offset + dt * CCH,
                ap=[vT_rep[:, :].ap[0], [P, BL]],
            ).bitcast(fp32)
            nc.vector.tensor_sub(verr[:, :], src, vr_slice)
            verr_rep = bass.AP(
                tensor=verr.tensor,
                offset=verr.offset,
                ap=[verr[:, :].ap[0], [1, BL], [0, P]],
            )
            nc.vector.tensor_copy(
                out=vT_err_rep[:, dt * CCH : (dt + 1) * CCH], in_=verr_rep
            )

        # ---- per-batch pipeline ---------------------------------------
        for bi in range(BL):
            cvt = []
            for dt in range(DT):
                t = cvpool.tile([P, C], f32r, tag="cv", name=f"cv{bi}_{dt}")
                # SWDGE cast-load: fp32 HBM -> f32r SBUF (rounding in DMA)
                nc.gpsimd.dma_start(
                    out=t[:, :], in_=cv_t[bi, dt * P : (dt + 1) * P, :]
                )
                cvt.append(t)

            # scores, partition-replicated: s[j][r, n] = sum_d v[d]*cv[d, n]
            s_ps = [
                psum.tile([P, CCH], fp32, tag="bank", name=f"s{bi}_{j}")
                for j in range(NJ)
            ]
            # keep each stationary operand loaded across 4 matmuls
            for dt in range(DT):
                lhsT = vT_rep[:, dt * CCH + bi * P : dt * CCH + (bi + 1) * P]
                lhsT_e = vT_err_rep[:, dt * CCH + bi * P : dt * CCH + (bi + 1) * P]
                for j in range(NJ):
                    nc.tensor.matmul(
                        s_ps[j][:, :], lhsT=lhsT,
                        rhs=cvt[dt][:, j * CCH : (j + 1) * CCH],
                        start=(dt == 0), stop=False,
                    )
                for j in range(NJ):
                    nc.tensor.matmul(
                        s_ps[j][:, :], lhsT=lhsT_e,
                        rhs=cvt[dt][:, j * CCH : (j + 1) * CCH],
                        start=False, stop=(dt == DT - 1),
                    )

            # softmax pieces (rows identical across partitions)
            m4 = small.tile([P, NJ], fp32, tag="m4")
            for j in range(NJ):
                nc.vector.reduce_max(
                    out=m4[:, j : j + 1], in_=s_ps[j][:, :], axis=AX.X
                )
            negm = small.tile([P, 1], fp32, tag="negm")
            nc.vector.reduce_max(out=negm[:, :], in_=m4[:, :], axis=AX.X, negate=True)

            p_sb = ppool.tile([P, C], fp32, tag="p")
            l4 = small.tile([P, NJ], fp32, tag="l4")
            for j in range(NJ):
                nc.scalar.activation(
                    out=p_sb[:, j * CCH : (j + 1) * CCH],
                    in_=s_ps[j][:, :],
                    func=AF.Exp,
                    bias=negm[:, :],
                    scale=1.0,
                    accum_out=l4[:, j : j + 1],
                )
            l1 = small.tile([P, 1], fp32, tag="l1")
            nc.vector.reduce_sum(out=l1[:, :], in_=l4[:, :], axis=AX.X)
            rl = small.tile([P, 1], fp32, tag="rl")
            nc.vector.reciprocal(out=rl[:, :], in_=l1[:, :])

            # pT_rep[c_lo, g*128 + r] = p[g*128 + c_lo] for all r
            # (transpose of the replicated p rows gives replicated columns)
            pT_rep = ptpool.tile([P, NG * P], f32r, tag="pt")
            for g4 in range(NG // 4):
                pt_ps = psum.tile([P, CCH], fp32, tag="bank", name=f"pt{bi}_{g4}")
                for gi in range(4):
                    g = g4 * 4 + gi
                    nc.tensor.transpose(
                        pt_ps[:, gi * P : (gi + 1) * P],
                        in_=p_sb[:, g * P : (g + 1) * P],
                        identity=ident_f[:, :],
                    )
                nc.vector.tensor_copy(
                    out=pT_rep[:, g4 * CCH : (g4 + 1) * CCH], in_=pt_ps[:, :]
                )

            # ctx (replicated): ctx[r, d] = sum_c cv[d, c] * p[c]
            ctx_ps = [
                psum.tile([P, CCH], fp32, tag="bank", name=f"ctx{bi}_{h}")
                for h in range(2)
            ]
            # per c-tile: all 8 transposes (one identity load), both copies,
            # then both matmuls (one pT load)
            for g in range(NG):
                ct_ps = []
                cvT_sb = []
                for h in range(2):
                    cp = psum.tile([P, CCH], f32r, tag="bank", name=f"ct{bi}_{g}_{h}")
                    for q in range(4):
                        dt = h * 4 + q
                        nc.tensor.transpose(
                            cp[:, q * P : (q + 1) * P],
                            in_=cvt[dt][:, g * P : (g + 1) * P],
                            identity=ident_r[:, :],
                        )
                    ct_ps.append(cp)
                for h in range(2):
                    sb = cvtpool.tile(
                        [P, CCH], f32r, tag="cvt", name=f"cvT{bi}_{g}_{h}"
                    )
                    if h == 0:
                        nc.vector.tensor_copy(out=sb[:, :], in_=ct_ps[h][:, :])
                    else:
                        nc.scalar.copy(out=sb[:, :], in_=ct_ps[h][:, :])
                    cvT_sb.append(sb)
                for h in range(2):
                    nc.tensor.matmul(
                        ctx_ps[h][:, :],
                        lhsT=pT_rep[:, g * P : (g + 1) * P],
                        rhs=cvT_sb[h][:, :],
                        start=(g == 0),
                        stop=(g == NG - 1),
                    )

            # normalize row 0 and store: out[t, bi, :] = ctx / l
            ctx_row = rowpool.tile([1, D], fp32, tag="crow")
            for h in range(2):
                nc.vector.tensor_scalar_mul(
                    ctx_row[:, h * CCH : (h + 1) * CCH],
                    ctx_ps[h][:1, :],
                    rl[:1, :],
                )
            ca = ctx_row[:, :]
            src_ap = bass.AP(
                tensor=ca.tensor,
                offset=ca.offset,
                ap=[ca.ap[0], [0, SEQ], [1, D]],
            )
            dst_ap = bass.AP(
                tensor=out_t,
                offset=bi * D,
                ap=[[0, 1], [BL * D, SEQ], [1, D]],
            )
            nc.sync.dma_start(out=dst_ap, in_=src_ap)

    if not nc.is_finalized():
        nc.finalize()
    return nc


def _get_nc():
    if "nc" not in _NC_CACHE:
        _NC_CACHE["nc"] = _build_nc()
    return _NC_CACHE["nc"]


def _make_in_maps(hidden, contextvects, W):
    ident = np.eye(P, dtype=np.float32)
    Wc = np.ascontiguousarray(W, dtype=np.float32)
    in_maps = []
    for k in range(N_CORES):
        sl = slice(k * BL, (k + 1) * BL)
        cv_local = np.ascontiguousarray(contextvects[sl], dtype=np.float32)
        hT_local = np.ascontiguousarray(hidden[0, sl, :].astype(np.float32).T)
        in_maps.append({"cv": cv_local, "hT": hT_local, "W": Wc, "ident": ident})
    return in_maps


def kernel(seqlen, hidden, contextvects, W, b, **_ignored):
    """Full-input entry point: shards across 8 NeuronCores internally."""
    from concourse.bass_utils import run_bass_kernel_spmd

    seqlen = int(seqlen)
    hidden = np.asarray(hidden)
    contextvects = np.asarray(contextvects)
    W = np.asarray(W)

    nc = _get_nc()
    in_maps = _make_in_maps(hidden, contextvects, W)
    res = run_bass_kernel_spmd(nc, in_maps, core_ids=list(range(N_CORES)))
    parts = [res.results[k]["out"] for k in range(N_CORES)]
    full = np.concatenate(parts, axis=1)
    if seqlen == SEQ:
        out = full
    else:
        out = np.broadcast_to(full[:1], (seqlen, B, D)).copy()
    return np.ascontiguousarray(out.astype(np.float32))
